# revision 49
# baseline (speedup 1.0000x reference)
"""Trainium2 Bass kernel for an AttentionBlock (LN -> QKV -> attn -> out-proj + residual).

Shapes (hardcoded per problem spec): B=8, L=1024, C=1024, H=8 heads.
The reference uses a raw row-major reshape (torch-style .view) of q/k/v from
[B, L, C] to [B*H, L, C/H]; with L=1024, C=1024, H=8 this makes each
"attention head" operate on a contiguous 128-sequence-row block of the
[L, C] matrix, reinterpreted as [1024, 128].

Sharding: pure data-parallel over batch, one batch element per NeuronCore
(8 cores). No collectives.

Precision strategy (tolerance is 2e-2 relative; the attention path only
contributes ~1% of the output magnitude, the LN residual dominates):
  - x input and the LN residual xn in bf16.
  - All big matmuls (QKV proj, softmax-denominator sums, attn@V, out-proj)
    in fp8 e4m3 with DoubleRow (contract 256 per instruction, 2x PE rate).
  - Scores matmul in bf16 (K=128 per head-chunk, DoubleRow not applicable).
Scales (fp8 max is +-240 on TRN):
  - xnT = 16 * xn (SX); weights = 256 * w (SW); proj psum = 4096 (SP)
  - qT/kT = 4096 * q_true (bias pre-scaled on host); exp scale = S2/SP^2
  - v_fp8 = 32 * v_true (SV); attnT = 32 * attn_true (SA = SV)
  - out psum = 32*256 * attn@w_out -> final scale 1/8192
b_qkv's v-bias is folded into b_out on the host (softmax weights sum to 1).

DMA: total bandwidth ~220GB/s shared across the three DMA queues and
~max(22ns, bytes/150GBps) per descriptor row per queue, so x and out are
repacked on the host to partition-major layouts with contiguous 2-4KB rows,
x tiles load first (split across sync+gpsimd), weights follow in priority
order (wv, wq, wk, wout), and the scalar queue is left free so the LN
activations are not delayed behind DMA dispatch instructions.
"""

import math
from contextlib import ExitStack

import ml_dtypes
import numpy as np

import concourse.bass as bass
import concourse.bacc as bacc
import concourse.tile as tile
from concourse import mybir
from concourse import bass_utils
from concourse.masks import make_identity

L = 1024
C = 1024
H = 8          # heads; also number of 128-row l-tiles (head h <-> l-tile h)
CH = 128       # head dim
NT = 8         # l tiles (128 rows each)
NG = 8         # c groups (128 cols each)
EPS = 1e-5
S2 = 1.0 / math.sqrt(CH)   # combined q&k scale: (ch^-0.25)^2

SX = 16.0      # xn -> fp8 scale
SW = 256.0     # weight -> fp8 scale
SP = SX * SW   # projection psum scale
SV = 32.0      # v_fp8 = SV * v_true
SA = 32.0      # attnT = SA * attn_true (== SV so rb = recip directly)
OUT_SCALE = 1.0 / (SA * SW)
EXP_SCALE = S2 / (SP * SP)
FP8_MAX = 240.0

f32 = mybir.dt.float32
bf16 = mybir.dt.bfloat16
fp8 = mybir.dt.float8e4
AF = mybir.ActivationFunctionType
ALU = mybir.AluOpType
DR = mybir.MatmulPerfMode.DoubleRow


def _bcast_ap(ap, p=128):
    """Broadcast a 1-D DRAM vector across p partitions (step-0 partition dim)."""
    return bass.AP(tensor=ap.tensor, offset=ap.offset, ap=[[0, p]] + list(ap.ap))


def _emit(nc, apply_affine: bool):
    # x and out are partition-major on the host: [p, t, c] with row l = 128t+p
    x_d = nc.dram_tensor("x", [128, NT * C], bf16, kind="ExternalInput").ap()
    wq_d = nc.dram_tensor("w_q", [128, NG * C], fp8, kind="ExternalInput").ap()
    wk_d = nc.dram_tensor("w_k", [128, NG * C], fp8, kind="ExternalInput").ap()
    wv_d = nc.dram_tensor("w_v", [128, NG * C], fp8, kind="ExternalInput").ap()
    bqk_d = nc.dram_tensor("b_qk", [128, 16], f32, kind="ExternalInput").ap()
    wout_d = nc.dram_tensor("w_out", [128, NG * C], fp8, kind="ExternalInput").ap()
    bout_d = nc.dram_tensor("b_out_eff", [C], f32, kind="ExternalInput").ap()
    if apply_affine:
        g_d = nc.dram_tensor("ln_g", [C], f32, kind="ExternalInput").ap()
        b_d = nc.dram_tensor("ln_b", [C], f32, kind="ExternalInput").ap()
    out_d = nc.dram_tensor("out", [128, NT * C], f32, kind="ExternalOutput").ap()

    with nc.allow_low_precision(reason="fp8/bf16 compute by design"), \
         tile.TileContext(nc) as tc, ExitStack() as ctx:
        const = ctx.enter_context(tc.tile_pool(name="const", bufs=1, side="left"))
        ident = const.tile([128, 128], bf16)
        make_identity(nc, ident)
        # DoubleRow lhsT needs the pair-dim stride to be a multiple of 16B
        ones2_t = const.tile([128, 2, 16], fp8)
        nc.vector.memset(ones2_t, 1.0)
        ones2 = ones2_t[:, :, 0:1]
        eps_sb = const.tile([128, 1], f32)
        nc.vector.memset(eps_sb, EPS)
        bqk_sb = const.tile([128, 16], f32)
        nc.sync.dma_start(out=bqk_sb[:], in_=bqk_d)
        bout_bc = const.tile([128, C], f32)
        if apply_affine:
            g_bc = const.tile([128, C], f32)
            b_bc = const.tile([128, C], f32)

        # Long-lived left-side tensors
        wqkv_pool = ctx.enter_context(tc.tile_pool(name="wqkv", bufs=1, side="left"))
        wv_sb = wqkv_pool.tile([128, NG, C], fp8)          # [c_in', k_in, d_v]
        wqk_sb = wqkv_pool.tile([128, NG, 2 * C], fp8)     # [c_in', k_in, d_qk]
        wout_pool = ctx.enter_context(tc.tile_pool(name="wout", bufs=1, side="left"))
        wout_sb = wout_pool.tile([128, NG, C], fp8)
        xt_pool = ctx.enter_context(tc.tile_pool(name="xt", bufs=1, side="left"))
        xt_all = xt_pool.tile([128, NT, C], bf16)   # raw x, [l_r, t, c]
        xn_pool = ctx.enter_context(tc.tile_pool(name="xn", bufs=1, side="left"))
        xn = xn_pool.tile([128, NT, C], bf16)       # normalized x
        xnb_pool = ctx.enter_context(tc.tile_pool(name="xnb", bufs=1, side="left"))
        xnb = xnb_pool.tile([128, NT, C], bf16)     # xn + b_out (residual+bias)
        xnT_pool = ctx.enter_context(tc.tile_pool(name="xnT", bufs=1, side="left"))
        xnT = xnT_pool.tile([128, NG, L], fp8)      # [c', g, l] = SX * xn.T
        attnT_pool = ctx.enter_context(tc.tile_pool(name="attnT", bufs=1, side="left"))
        attnT = attnT_pool.tile([128, NG, L], fp8)  # [c', g_q, l] = SA * attn.T
        v_pool = tc.alloc_tile_pool(name="v", bufs=1, side="left")
        v_fp8 = v_pool.tile([128, NT, C], fp8)      # [l_r, l-tile, c] = SV * v
        qT_pool = tc.alloc_tile_pool(name="qT", bufs=1, side="left")
        qT = qT_pool.tile([128, H, NG, 128], bf16)  # [c', h, g_q, l_r]
        kT_pool = tc.alloc_tile_pool(name="kT", bufs=1, side="left")
        kT = kT_pool.tile([128, NG, L], bf16)       # [c', g_k, l]

        # DMA plan: x tiles first (sync+gpsimd), then weights in priority
        # order split in k-halves across the same two queues. The scalar
        # queue carries no DMAs so LN activations issue immediately.
        xt_flat = xt_all[:].rearrange("p t c -> p (t c)")
        # first two tiles in halves across both queues for a faster LN start
        for i in range(4):
            eng = nc.sync if i % 2 == 0 else nc.gpsimd
            eng.dma_start(out=xt_flat[:, 512 * i:512 * (i + 1)],
                          in_=x_d[:, 512 * i:512 * (i + 1)])
        for t in range(2, NT):
            eng = nc.sync if t % 2 == 0 else nc.gpsimd
            eng.dma_start(out=xt_flat[:, C * t:C * (t + 1)],
                          in_=x_d[:, C * t:C * (t + 1)])

        def _load_w_halves(sb, dram, n):
            for half, eng in ((0, nc.sync), (1, nc.gpsimd)):
                eng.dma_start(
                    out=sb[:, 4 * half:4 * (half + 1), :],
                    in_=dram[:, 4 * n * half:4 * n * (half + 1)].rearrange(
                        "p (k n) -> p k n", k=4))

        _load_w_halves(wv_sb, wv_d, C)
        _load_w_halves(wqk_sb[:, :, 0:C], wq_d, C)
        _load_w_halves(wqk_sb[:, :, C:2 * C], wk_d, C)
        _load_w_halves(wout_sb, wout_d, C)
        if apply_affine:
            nc.gpsimd.dma_start(out=g_bc[:], in_=_bcast_ap(g_d))
            nc.gpsimd.dma_start(out=b_bc[:], in_=_bcast_ap(b_d))
        nc.gpsimd.dma_start(out=bout_bc[:], in_=_bcast_ap(bout_d))

        # ---------- Phase 1: per-tile LayerNorm + transpose to xnT ----------
        with tc.tile_pool(name="lnst", bufs=4, side="right") as lnst, \
             tc.tile_pool(name="lntmp", bufs=3, side="right") as lntmp, \
             tc.tile_pool(name="tr_ps", bufs=2, space="PSUM") as tr_ps, \
             tc.tile_pool(name="proj_ps", bufs=2, space="PSUM") as proj_ps:
            for t in range(NT):
                xt = xt_all[:, t, :]
                stats = lnst.tile([128, 2, 6], f32)
                for j in range(2):
                    nc.vector.bn_stats(out=stats[:, j, :],
                                       in_=xt[:, 512 * j:512 * (j + 1)])
                mv = lnst.tile([128, 2], f32)
                nc.vector.bn_aggr(out=mv[:], in_=stats[:])
                sq = lnst.tile([128, 1], f32)
                nc.scalar.activation(out=sq[:], in_=mv[:, 1:2], func=AF.Sqrt,
                                     bias=eps_sb[:], scale=1.0)
                rstd = lnst.tile([128, 1], f32)
                nc.vector.reciprocal(out=rstd[:], in_=sq[:])
                nmr = lnst.tile([128, 1], f32)
                nc.vector.tensor_scalar(nmr[:], mv[:, 0:1], rstd[:], -1.0,
                                        ALU.mult, ALU.mult)
                if apply_affine:
                    zt = lntmp.tile([128, C], f32)
                    nc.scalar.activation(out=zt[:], in_=xt, func=AF.Identity,
                                         bias=nmr[:], scale=rstd[:])
                    zg = lntmp.tile([128, C], f32)
                    nc.vector.tensor_tensor(out=zg[:], in0=zt[:], in1=g_bc[:],
                                            op=ALU.mult)
                    nc.vector.tensor_tensor(out=xn[:, t, :], in0=zg[:], in1=b_bc[:],
                                            op=ALU.add)
                else:
                    nc.scalar.activation(out=xn[:, t, :], in_=xt, func=AF.Identity,
                                         bias=nmr[:], scale=rstd[:])
                # transpose tile t: 8 PE transposes into one PSUM bank, then a
                # single DVE copy (x SX, cast to fp8) into xnT
                trp = tr_ps.tile([128, NG, 128], bf16, tag="tr")
                for g in range(NG):
                    nc.tensor.transpose(trp[:, g, :], xn[:, t, 128 * g:128 * (g + 1)],
                                        ident[:])
                nc.vector.tensor_scalar(
                    xnT[:, :, 128 * t:128 * (t + 1)], trp[:], SX, None, ALU.mult)

            # ---------- Phase 2: V projection (DoubleRow fp8) ----------
            for m in range(NT):
                psv = proj_ps.tile([128, C], f32, tag="proj")
                for kp in range(4):
                    lhsT = xnT[:, 2 * kp:2 * kp + 2, 128 * m:128 * (m + 1)]
                    for j in range(2):
                        nc.tensor.matmul(
                            psv[:, 512 * j:512 * (j + 1)], lhsT,
                            wv_sb[:, 2 * kp:2 * kp + 2, 512 * j:512 * (j + 1)],
                            start=(kp == 0), stop=(kp == 3), perf_mode=DR)
                nc.scalar.activation(out=v_fp8[:, m, :], in_=psv[:],
                                     func=AF.Identity, bias=0.0, scale=SV / SP)

            # ---------- Phase 3: Q, K projections (DoubleRow fp8) ----------
            for co in range(16):
                psq = proj_ps.tile([128, L], f32, tag="proj")
                for kp in range(4):
                    lhsT = wqk_sb[:, 2 * kp:2 * kp + 2, 128 * co:128 * (co + 1)]
                    for j in range(2):
                        nc.tensor.matmul(
                            psq[:, 512 * j:512 * (j + 1)], lhsT,
                            xnT[:, 2 * kp:2 * kp + 2, 512 * j:512 * (j + 1)],
                            start=(kp == 0), stop=(kp == 3), perf_mode=DR)
                bias_col = bqk_sb[:, co:co + 1]
                if co < 8:
                    # q: dst [c', h, l_r] over h (l = 128h + l_r); ScalarE
                    nc.scalar.activation(
                        out=qT[:, :, co, :],
                        in_=psq[:].rearrange("p (h l) -> p h l", h=H),
                        func=AF.Identity, bias=bias_col, scale=1.0)
                else:
                    # k: DVE
                    nc.vector.tensor_scalar(kT[:, co - 8, :], psq[:],
                                            bias_col, None, ALU.add)

            # residual + out-bias, precomputed so the out-proj drain is 2 ops
            for t in range(NT):
                nc.vector.tensor_tensor(out=xnb[:, t, :], in0=xn[:, t, :],
                                        in1=bout_bc[:], op=ALU.add)

        # ---------- Phase 4: attention + out-proj, software-pipelined ----------
        # stage lag: scores/exp/sums(h) | attnV(h-1) | out-proj(h-2)
        # PSUM budget (8 banks): scores 3x[128,512]=3, sums [1,L]=2,
        # attnV [128,L]=2, out-proj half [128,512]=1.
        with tc.tile_pool(name="pt", bufs=3, side="right") as pt_pool, \
             tc.tile_pool(name="rb", bufs=3, side="right") as rb_pool, \
             tc.tile_pool(name="recip", bufs=3, side="right") as recip_pool, \
             tc.tile_pool(name="ot", bufs=3, side="right") as ot_pool, \
             tc.tile_pool(name="s_ps", bufs=3, space="PSUM") as s_ps, \
             tc.tile_pool(name="sum_ps", bufs=1, space="PSUM") as sum_ps, \
             tc.tile_pool(name="av_ps", bufs=1, space="PSUM") as av_ps, \
             tc.tile_pool(name="o_ps", bufs=1, space="PSUM") as o_ps:

            state = {}   # per-head tiles carried across pipeline stages

            def emit_scores_start(h):
                pt = pt_pool.tile([128, NG, L], fp8, name=f"pt{h}", tag="pt")
                ps_sum = sum_ps.tile([1, L], f32, name=f"psum{h}", tag="ps_sum")
                state[h] = {"pt": pt, "ps_sum": ps_sum}

            def emit_scores_gk(h, gk):
                st = state[h]
                hs = slice(128 * h, 128 * (h + 1))
                qrow = qT[:, h, :, :].rearrange("p g l -> p (g l)")
                for j in range(2):
                    ps_s = s_ps.tile([128, 512], f32, tag="ps_s")
                    nc.tensor.matmul(ps_s[:], kT[:, gk, hs],
                                     qrow[:, 512 * j:512 * (j + 1)],
                                     start=True, stop=True)
                    nc.scalar.activation(
                        out=st["pt"][:, gk, 512 * j:512 * (j + 1)], in_=ps_s[:],
                        func=AF.Exp, bias=0.0, scale=EXP_SCALE)

            def emit_sums_pair(h, p):
                st = state[h]
                pt, ps_sum = st["pt"], st["ps_sum"]
                for j in range(2):
                    nc.tensor.matmul(
                        ps_sum[:, 512 * j:512 * (j + 1)], ones2,
                        pt[:, 2 * p:2 * p + 2, 512 * j:512 * (j + 1)],
                        start=(p == 0), stop=(p == 3), perf_mode=DR)

            def emit_recip(h):
                st = state[h]
                recip = recip_pool.tile([1, L], f32, tag="recip")
                nc.vector.reciprocal_approx_fast(out=recip[:], in_=st["ps_sum"])
                rb = rb_pool.tile([128, L], f32, tag="rb")
                nc.gpsimd.partition_broadcast(rb[:], recip[:])
                st["rb"] = rb

            def emit_attnv_mm(h, i):
                # i in 0..7 -> (p, j)
                st = state[h]
                p, j = divmod(i, 2)
                if i == 0:
                    st["av"] = av_ps.tile([128, L], f32, name=f"av{h}",
                                          tag="ps_av")
                vrow = v_fp8[:, h, :].rearrange("p (g c) -> p g c", g=NG)
                nc.tensor.matmul(
                    st["av"][:, 512 * j:512 * (j + 1)],
                    vrow[:, 2 * p:2 * p + 2, :],
                    st["pt"][:, 2 * p:2 * p + 2, 512 * j:512 * (j + 1)],
                    start=(p == 0), stop=(p == 3), perf_mode=DR)

            def emit_attnv_done(h):
                st = state[h]
                hs = slice(128 * h, 128 * (h + 1))
                nc.vector.tensor_tensor(
                    out=attnT[:, :, hs],
                    in0=st["av"][:].rearrange("p (g l) -> p g l", g=NG),
                    in1=st["rb"][:].rearrange("p (g l) -> p g l", g=NG),
                    op=ALU.mult)

            def emit_outproj_mm(h, i):
                # i in 0..7 -> (j, kp): j-major so each half finishes early
                st = state[h]
                j, kp = divmod(i, 4)
                if kp == 0:
                    st[f"po{j}"] = o_ps.tile([128, 512], f32, name=f"po{h}_{j}",
                                             tag="ps_o")
                lhsT = attnT[:, 2 * kp:2 * kp + 2, 128 * h:128 * (h + 1)]
                nc.tensor.matmul(
                    st[f"po{j}"][:],
                    lhsT,
                    wout_sb[:, 2 * kp:2 * kp + 2, 512 * j:512 * (j + 1)],
                    start=(kp == 0), stop=(kp == 3), perf_mode=DR)

            def emit_outproj_drain(h, j):
                st = state[h]
                if j == 0:
                    st["t3"] = ot_pool.tile([128, C], f32, name=f"t3_{h}",
                                            tag="ot")
                t1 = ot_pool.tile([128, 512], f32, tag="ot")
                nc.vector.tensor_scalar(t1[:], st[f"po{j}"][:], OUT_SCALE, None,
                                        ALU.mult)
                nc.vector.tensor_tensor(
                    out=st["t3"][:, 512 * j:512 * (j + 1)], in0=t1[:],
                    in1=xnb[:, h, 512 * j:512 * (j + 1)], op=ALU.add)
                if j == 1:
                    if h == H - 1:
                        # last tile: split across both queues to shorten the tail
                        nc.sync.dma_start(out=out_d[:, C * h:C * h + 512],
                                          in_=st["t3"][:, 0:512])
                        nc.gpsimd.dma_start(out=out_d[:, C * h + 512:C * (h + 1)],
                                            in_=st["t3"][:, 512:1024])
                    else:
                        eng = nc.sync if h % 2 == 0 else nc.gpsimd
                        eng.dma_start(out=out_d[:, C * h:C * (h + 1)],
                                      in_=st["t3"][:])
                    del state[h]

            # pipeline: head h does scores; h-1 attnV; h-2 out-proj
            for h in range(H + 2):
                if h < H:
                    emit_scores_start(h)
                for gk in range(NG):
                    if h < H:
                        emit_scores_gk(h, gk)
                    if h >= 1 and h - 1 < H:
                        emit_attnv_mm(h - 1, gk)
                    if h >= 2 and h - 2 < H:
                        emit_outproj_mm(h - 2, gk)
                        if gk == 3:
                            emit_outproj_drain(h - 2, 0)
                    # sums for pair p at gk = 2p+3 (exp of pair done ~1 iter ago)
                    if h < H and gk >= 3 and gk % 2 == 1:
                        emit_sums_pair(h, (gk - 3) // 2)
                if h < H:
                    emit_sums_pair(h, 3)
                # attnT-mult of h-1 goes on the DVE queue BEFORE recip(h):
                # it frees the single attnV PSUM buffer that head h's first
                # attnV matmul is about to wait on
                if h >= 1 and h - 1 < H:
                    emit_attnv_done(h - 1)
                if h < H:
                    emit_recip(h)
                if h >= 2 and h - 2 < H:
                    emit_outproj_drain(h - 2, 1)

        kT_pool.release()
        qT_pool.release()
        v_pool.release()

    return nc


_CACHE = {}


def _build(apply_affine: bool):
    key = apply_affine
    if key not in _CACHE:
        nc = bacc.Bacc("TRN2", target_bir_lowering=False, debug=False)
        _emit(nc, apply_affine)
        nc.compile()
        _CACHE[key] = nc
    return _CACHE[key]


def _to_fp8(a: np.ndarray) -> np.ndarray:
    return np.ascontiguousarray(
        np.clip(a, -FP8_MAX, FP8_MAX).astype(ml_dtypes.float8_e4m3))


def _pkn(w):
    """[C, n] row-major -> partition-major [128, (k, n)] with row c = 128k+p."""
    n = w.shape[1]
    return np.ascontiguousarray(
        w.reshape(NG, 128, n).transpose(1, 0, 2).reshape(128, NG * n))


def make_in_maps(inputs, B: int, apply_affine: bool):
    x = np.asarray(inputs["x"], np.float32)
    ln_g = np.asarray(inputs["ln_g"], np.float32)
    ln_b = np.asarray(inputs["ln_b"], np.float32)
    w_qkv = np.ascontiguousarray(np.asarray(inputs["w_qkv"], np.float32))
    b_qkv = np.asarray(inputs["b_qkv"], np.float32)
    w_out = np.ascontiguousarray(np.asarray(inputs["w_out"], np.float32))
    b_out = np.asarray(inputs["b_out"], np.float32)

    wqkv_fp8 = _to_fp8(w_qkv * SW)
    wq_pre = _pkn(wqkv_fp8[:, :C])
    wk_pre = _pkn(wqkv_fp8[:, C:2 * C])
    wv_pre = _pkn(wqkv_fp8[:, 2 * C:])
    wout_pre = _pkn(_to_fp8(w_out * SW))
    # q/k biases pre-scaled by SP (psum scale); [128, 16] col-per-slab
    bqk_pre = np.ascontiguousarray(
        (b_qkv[:2 * C] * SP).reshape(16, 128).T.astype(np.float32))
    # v-bias folded into out bias: softmax rows sum to 1
    b_out_eff = (b_out.astype(np.float64)
                 + b_qkv[2 * C:].astype(np.float64) @ w_out.astype(np.float64)
                 ).astype(np.float32)

    x_bf = x.astype(ml_dtypes.bfloat16)
    in_maps = []
    for c in range(B):
        m = {
            "x": _pkn(x_bf[c]),
            "w_q": wq_pre,
            "w_k": wk_pre,
            "w_v": wv_pre,
            "b_qk": bqk_pre,
            "w_out": wout_pre,
            "b_out_eff": b_out_eff,
        }
        if apply_affine:
            m["ln_g"] = ln_g
            m["ln_b"] = ln_b
        in_maps.append(m)
    return in_maps


def kernel(**inputs) -> np.ndarray:
    x = np.asarray(inputs["x"], np.float32)
    ln_g = np.asarray(inputs["ln_g"], np.float32)
    ln_b = np.asarray(inputs["ln_b"], np.float32)
    B = x.shape[0]
    assert x.shape == (B, L, C)
    apply_affine = not (np.all(ln_g == 1.0) and np.all(ln_b == 0.0))
    nc = _build(apply_affine)
    in_maps = make_in_maps(inputs, B, apply_affine)
    res = bass_utils.run_bass_kernel_spmd(nc, in_maps, core_ids=list(range(B)))
    # out is partition-major [128, (t, c)]: row l = 128t + p
    return np.stack([
        res.results[c]["out"].reshape(128, NT, C).transpose(1, 0, 2)
        .reshape(L, C) for c in range(B)
    ]).astype(np.float32)


# revision 51
# speedup vs baseline: 1.1879x; 1.1879x over previous
"""Trainium2 Bass kernel for an AttentionBlock (LN -> QKV -> attn -> out-proj + residual).

Shapes (hardcoded per problem spec): B=8, L=1024, C=1024, H=8 heads.
The reference uses a raw row-major reshape (torch-style .view) of q/k/v from
[B, L, C] to [B*H, L, C/H]; with L=1024, C=1024, H=8 this makes each
"attention head" operate on a contiguous 128-sequence-row block of the
[L, C] matrix, reinterpreted as [1024, 128].

Sharding: pure data-parallel over batch, one batch element per NeuronCore
(8 cores). No collectives.

Precision strategy (tolerance is 2e-2 relative; the attention path only
contributes ~1% of the output magnitude, the LN residual dominates):
  - x input and the LN residual xn in bf16.
  - All big matmuls (QKV proj, softmax-denominator sums, attn@V, out-proj)
    in fp8 e4m3 with DoubleRow (contract 256 per instruction, 2x PE rate).
  - Scores matmul in bf16 (K=128 per head-chunk, DoubleRow not applicable).
Scales (fp8 max is +-240 on TRN):
  - xnT = 16 * xn (SX); weights = 256 * w (SW); proj psum = 4096 (SP)
  - qT/kT = 4096 * q_true (bias pre-scaled on host); exp scale = S2/SP^2
  - v_fp8 = 32 * v_true (SV); attnT = 32 * attn_true (SA = SV)
  - out psum = 32*256 * attn@w_out -> final scale 1/8192
b_qkv's v-bias is folded into b_out on the host (softmax weights sum to 1).

DMA: total bandwidth ~220GB/s shared across the three DMA queues and
~max(22ns, bytes/150GBps) per descriptor row per queue, so x and out are
repacked on the host to partition-major layouts with contiguous 2-4KB rows,
x tiles load first (split across sync+gpsimd), weights follow in priority
order (wv, wq, wk, wout), and the scalar queue is left free so the LN
activations are not delayed behind DMA dispatch instructions.
"""

import math
from contextlib import ExitStack

import ml_dtypes
import numpy as np

import concourse.bass as bass
import concourse.bacc as bacc
import concourse.tile as tile
from concourse import mybir
from concourse import bass_utils
from concourse.masks import make_identity

L = 1024
C = 1024
H = 8          # heads; also number of 128-row l-tiles (head h <-> l-tile h)
CH = 128       # head dim
NT = 8         # l tiles (128 rows each)
NG = 8         # c groups (128 cols each)
EPS = 1e-5
S2 = 1.0 / math.sqrt(CH)   # combined q&k scale: (ch^-0.25)^2

SX = 16.0      # xn -> fp8 scale
SW = 256.0     # weight -> fp8 scale
SP = SX * SW   # projection psum scale
SV = 32.0      # v_fp8 = SV * v_true
SA = 32.0      # attnT = SA * attn_true (== SV so rb = recip directly)
OUT_SCALE = 1.0 / (SA * SW)
EXP_SCALE = S2 / (SP * SP)
FP8_MAX = 240.0

f32 = mybir.dt.float32
bf16 = mybir.dt.bfloat16
fp8 = mybir.dt.float8e4
AF = mybir.ActivationFunctionType
ALU = mybir.AluOpType
DR = mybir.MatmulPerfMode.DoubleRow


def _bcast_ap(ap, p=128):
    """Broadcast a 1-D DRAM vector across p partitions (step-0 partition dim)."""
    return bass.AP(tensor=ap.tensor, offset=ap.offset, ap=[[0, p]] + list(ap.ap))


def _emit(nc, apply_affine: bool):
    # x and out are partition-major on the host: [p, t, c] with row l = 128t+p
    x_d = nc.dram_tensor("x", [128, NT * C], bf16, kind="ExternalInput").ap()
    wq_d = nc.dram_tensor("w_q", [128, NG * C], fp8, kind="ExternalInput").ap()
    wk_d = nc.dram_tensor("w_k", [128, NG * C], fp8, kind="ExternalInput").ap()
    wv_d = nc.dram_tensor("w_v", [128, NG * C], fp8, kind="ExternalInput").ap()
    bqk_d = nc.dram_tensor("b_qk", [128, 16], f32, kind="ExternalInput").ap()
    wout_d = nc.dram_tensor("w_out", [128, NG * C], fp8, kind="ExternalInput").ap()
    bout_d = nc.dram_tensor("b_out_eff", [C], f32, kind="ExternalInput").ap()
    if apply_affine:
        g_d = nc.dram_tensor("ln_g", [C], f32, kind="ExternalInput").ap()
        b_d = nc.dram_tensor("ln_b", [C], f32, kind="ExternalInput").ap()
    out_d = nc.dram_tensor("out", [128, NT * C], f32, kind="ExternalOutput").ap()

    with nc.allow_low_precision(reason="fp8/bf16 compute by design"), \
         tile.TileContext(nc) as tc, ExitStack() as ctx:
        const = ctx.enter_context(tc.tile_pool(name="const", bufs=1, side="left"))
        ident = const.tile([128, 128], bf16)
        make_identity(nc, ident)
        # DoubleRow lhsT needs the pair-dim stride to be a multiple of 16B
        ones2_t = const.tile([128, 2, 16], fp8)
        nc.vector.memset(ones2_t, 1.0)
        ones2 = ones2_t[:, :, 0:1]
        eps_sb = const.tile([128, 1], f32)
        nc.vector.memset(eps_sb, EPS)
        bqk_sb = const.tile([128, 16], f32)
        nc.sync.dma_start(out=bqk_sb[:], in_=bqk_d)
        bout_bc = const.tile([128, C], f32)
        if apply_affine:
            g_bc = const.tile([128, C], f32)
            b_bc = const.tile([128, C], f32)

        # Long-lived left-side tensors
        wqkv_pool = ctx.enter_context(tc.tile_pool(name="wqkv", bufs=1, side="left"))
        wv_sb = wqkv_pool.tile([128, NG, C], fp8)          # [c_in', k_in, d_v]
        wqk_sb = wqkv_pool.tile([128, NG, 2 * C], fp8)     # [c_in', k_in, d_qk]
        wout_pool = ctx.enter_context(tc.tile_pool(name="wout", bufs=1, side="left"))
        wout_sb = wout_pool.tile([128, NG, C], fp8)
        xt_pool = ctx.enter_context(tc.tile_pool(name="xt", bufs=1, side="left"))
        xt_all = xt_pool.tile([128, NT, C], bf16)   # raw x, [l_r, t, c]
        xn_pool = ctx.enter_context(tc.tile_pool(name="xn", bufs=1, side="left"))
        xn = xn_pool.tile([128, NT, C], bf16)       # normalized x
        xnb_pool = ctx.enter_context(tc.tile_pool(name="xnb", bufs=1, side="left"))
        xnb = xnb_pool.tile([128, NT, C], bf16)     # xn + b_out (residual+bias)
        xnT_pool = ctx.enter_context(tc.tile_pool(name="xnT", bufs=1, side="left"))
        xnT = xnT_pool.tile([128, NG, L], fp8)      # [c', g, l] = SX * xn.T
        attnT_pool = ctx.enter_context(tc.tile_pool(name="attnT", bufs=1, side="left"))
        attnT = attnT_pool.tile([128, NG, L], fp8)  # [c', g_q, l] = SA * attn.T
        v_pool = tc.alloc_tile_pool(name="v", bufs=1, side="left")
        v_fp8 = v_pool.tile([128, NT, C], fp8)      # [l_r, l-tile, c] = SV * v
        qT_pool = tc.alloc_tile_pool(name="qT", bufs=1, side="left")
        qT = qT_pool.tile([128, H, NG, 128], bf16)  # [c', h, g_q, l_r]
        kT_pool = tc.alloc_tile_pool(name="kT", bufs=1, side="left")
        kT = kT_pool.tile([128, NG, L], bf16)       # [c', g_k, l]

        # DMA plan: x tiles first (sync+gpsimd), then weights in priority
        # order split in k-halves across the same two queues. The scalar
        # queue carries no DMAs so LN activations issue immediately.
        xt_flat = xt_all[:].rearrange("p t c -> p (t c)")
        for t in range(NT):
            eng = nc.sync if t % 2 == 0 else nc.gpsimd
            eng.dma_start(out=xt_flat[:, C * t:C * (t + 1)],
                          in_=x_d[:, C * t:C * (t + 1)])

        def _load_w_halves(sb, dram, n):
            for half, eng in ((0, nc.sync), (1, nc.gpsimd)):
                eng.dma_start(
                    out=sb[:, 4 * half:4 * (half + 1), :],
                    in_=dram[:, 4 * n * half:4 * n * (half + 1)].rearrange(
                        "p (k n) -> p k n", k=4))

        _load_w_halves(wv_sb, wv_d, C)
        _load_w_halves(wqk_sb[:, :, 0:C], wq_d, C)
        _load_w_halves(wqk_sb[:, :, C:2 * C], wk_d, C)
        _load_w_halves(wout_sb, wout_d, C)
        if apply_affine:
            nc.gpsimd.dma_start(out=g_bc[:], in_=_bcast_ap(g_d))
            nc.gpsimd.dma_start(out=b_bc[:], in_=_bcast_ap(b_d))
        nc.gpsimd.dma_start(out=bout_bc[:], in_=_bcast_ap(bout_d))

        # ---------- Phase 1: per-tile LayerNorm + transpose to xnT ----------
        with tc.tile_pool(name="lnst", bufs=4, side="right") as lnst, \
             tc.tile_pool(name="lntmp", bufs=3, side="right") as lntmp, \
             tc.tile_pool(name="tr_ps", bufs=2, space="PSUM") as tr_ps, \
             tc.tile_pool(name="proj_ps", bufs=2, space="PSUM") as proj_ps:
            for t in range(NT):
                xt = xt_all[:, t, :]
                stats = lnst.tile([128, 2, 6], f32)
                for j in range(2):
                    nc.vector.bn_stats(out=stats[:, j, :],
                                       in_=xt[:, 512 * j:512 * (j + 1)])
                mv = lnst.tile([128, 2], f32)
                nc.vector.bn_aggr(out=mv[:], in_=stats[:])
                sq = lnst.tile([128, 1], f32)
                nc.scalar.activation(out=sq[:], in_=mv[:, 1:2], func=AF.Sqrt,
                                     bias=eps_sb[:], scale=1.0)
                rstd = lnst.tile([128, 1], f32)
                nc.vector.reciprocal(out=rstd[:], in_=sq[:])
                nmr = lnst.tile([128, 1], f32)
                nc.vector.tensor_scalar(nmr[:], mv[:, 0:1], rstd[:], -1.0,
                                        ALU.mult, ALU.mult)
                if apply_affine:
                    zt = lntmp.tile([128, C], f32)
                    nc.scalar.activation(out=zt[:], in_=xt, func=AF.Identity,
                                         bias=nmr[:], scale=rstd[:])
                    zg = lntmp.tile([128, C], f32)
                    nc.vector.tensor_tensor(out=zg[:], in0=zt[:], in1=g_bc[:],
                                            op=ALU.mult)
                    nc.vector.tensor_tensor(out=xn[:, t, :], in0=zg[:], in1=b_bc[:],
                                            op=ALU.add)
                else:
                    nc.scalar.activation(out=xn[:, t, :], in_=xt, func=AF.Identity,
                                         bias=nmr[:], scale=rstd[:])
                # transpose tile t: 8 PE transposes into one PSUM bank, then a
                # single DVE copy (x SX, cast to fp8) into xnT
                trp = tr_ps.tile([128, NG, 128], bf16, tag="tr")
                for g in range(NG):
                    nc.tensor.transpose(trp[:, g, :], xn[:, t, 128 * g:128 * (g + 1)],
                                        ident[:])
                nc.vector.tensor_scalar(
                    xnT[:, :, 128 * t:128 * (t + 1)], trp[:], SX, None, ALU.mult)

            # ---------- Phase 2: V projection (DoubleRow fp8) ----------
            for m in range(NT):
                psv = proj_ps.tile([128, C], f32, tag="proj")
                for kp in range(4):
                    lhsT = xnT[:, 2 * kp:2 * kp + 2, 128 * m:128 * (m + 1)]
                    for j in range(2):
                        nc.tensor.matmul(
                            psv[:, 512 * j:512 * (j + 1)], lhsT,
                            wv_sb[:, 2 * kp:2 * kp + 2, 512 * j:512 * (j + 1)],
                            start=(kp == 0), stop=(kp == 3), perf_mode=DR)
                nc.scalar.activation(out=v_fp8[:, m, :], in_=psv[:],
                                     func=AF.Identity, bias=0.0, scale=SV / SP)

            # ---------- Phase 3: Q, K projections (DoubleRow fp8) ----------
            for co in range(16):
                psq = proj_ps.tile([128, L], f32, tag="proj")
                for kp in range(4):
                    lhsT = wqk_sb[:, 2 * kp:2 * kp + 2, 128 * co:128 * (co + 1)]
                    for j in range(2):
                        nc.tensor.matmul(
                            psq[:, 512 * j:512 * (j + 1)], lhsT,
                            xnT[:, 2 * kp:2 * kp + 2, 512 * j:512 * (j + 1)],
                            start=(kp == 0), stop=(kp == 3), perf_mode=DR)
                bias_col = bqk_sb[:, co:co + 1]
                if co < 8:
                    # q: dst [c', h, l_r] over h (l = 128h + l_r); ScalarE
                    nc.scalar.activation(
                        out=qT[:, :, co, :],
                        in_=psq[:].rearrange("p (h l) -> p h l", h=H),
                        func=AF.Identity, bias=bias_col, scale=1.0)
                else:
                    # k: DVE
                    nc.vector.tensor_scalar(kT[:, co - 8, :], psq[:],
                                            bias_col, None, ALU.add)

            # residual + out-bias, precomputed so the out-proj drain is 2 ops
            for t in range(NT):
                nc.vector.tensor_tensor(out=xnb[:, t, :], in0=xn[:, t, :],
                                        in1=bout_bc[:], op=ALU.add)

        # ---------- Phase 4: attention + out-proj, software-pipelined ----------
        # stage lag: scores/exp/sums(h) | attnV(h-1) | out-proj(h-2)
        # PSUM budget (8 banks): scores 3x[128,512]=3, sums [1,L]=2,
        # attnV [128,L]=2, out-proj half [128,512]=1.
        with tc.tile_pool(name="pt", bufs=3, side="right") as pt_pool, \
             tc.tile_pool(name="rb", bufs=3, side="right") as rb_pool, \
             tc.tile_pool(name="recip", bufs=3, side="right") as recip_pool, \
             tc.tile_pool(name="ot", bufs=3, side="right") as ot_pool, \
             tc.tile_pool(name="s_ps", bufs=3, space="PSUM") as s_ps, \
             tc.tile_pool(name="sum_ps", bufs=1, space="PSUM") as sum_ps, \
             tc.tile_pool(name="av_ps", bufs=1, space="PSUM") as av_ps, \
             tc.tile_pool(name="o_ps", bufs=1, space="PSUM") as o_ps:

            state = {}   # per-head tiles carried across pipeline stages

            def emit_scores_start(h):
                pt = pt_pool.tile([128, NG, L], fp8, name=f"pt{h}", tag="pt")
                ps_sum = sum_ps.tile([1, L], f32, name=f"psum{h}", tag="ps_sum")
                state[h] = {"pt": pt, "ps_sum": ps_sum}

            def emit_scores_gk(h, gk):
                st = state[h]
                hs = slice(128 * h, 128 * (h + 1))
                qrow = qT[:, h, :, :].rearrange("p g l -> p (g l)")
                for j in range(2):
                    ps_s = s_ps.tile([128, 512], f32, tag="ps_s")
                    nc.tensor.matmul(ps_s[:], kT[:, gk, hs],
                                     qrow[:, 512 * j:512 * (j + 1)],
                                     start=True, stop=True)
                    nc.scalar.activation(
                        out=st["pt"][:, gk, 512 * j:512 * (j + 1)], in_=ps_s[:],
                        func=AF.Exp, bias=0.0, scale=EXP_SCALE)

            def emit_sums_pair(h, p):
                st = state[h]
                pt, ps_sum = st["pt"], st["ps_sum"]
                for j in range(2):
                    nc.tensor.matmul(
                        ps_sum[:, 512 * j:512 * (j + 1)], ones2,
                        pt[:, 2 * p:2 * p + 2, 512 * j:512 * (j + 1)],
                        start=(p == 0), stop=(p == 3), perf_mode=DR)

            def emit_recip(h):
                st = state[h]
                recip = recip_pool.tile([1, L], f32, tag="recip")
                nc.vector.reciprocal_approx_fast(out=recip[:], in_=st["ps_sum"])
                rb = rb_pool.tile([128, L], f32, tag="rb")
                nc.gpsimd.partition_broadcast(rb[:], recip[:])
                st["rb"] = rb

            def emit_attnv_mm(h, i):
                # i in 0..7 -> (p, j)
                st = state[h]
                p, j = divmod(i, 2)
                if i == 0:
                    st["av"] = av_ps.tile([128, L], f32, name=f"av{h}",
                                          tag="ps_av")
                vrow = v_fp8[:, h, :].rearrange("p (g c) -> p g c", g=NG)
                nc.tensor.matmul(
                    st["av"][:, 512 * j:512 * (j + 1)],
                    vrow[:, 2 * p:2 * p + 2, :],
                    st["pt"][:, 2 * p:2 * p + 2, 512 * j:512 * (j + 1)],
                    start=(p == 0), stop=(p == 3), perf_mode=DR)

            def emit_attnv_done(h):
                st = state[h]
                hs = slice(128 * h, 128 * (h + 1))
                nc.vector.tensor_tensor(
                    out=attnT[:, :, hs],
                    in0=st["av"][:].rearrange("p (g l) -> p g l", g=NG),
                    in1=st["rb"][:].rearrange("p (g l) -> p g l", g=NG),
                    op=ALU.mult)

            def emit_outproj_mm(h, i):
                # i in 0..7 -> (j, kp): j-major so each half finishes early
                st = state[h]
                j, kp = divmod(i, 4)
                if kp == 0:
                    st[f"po{j}"] = o_ps.tile([128, 512], f32, name=f"po{h}_{j}",
                                             tag="ps_o")
                lhsT = attnT[:, 2 * kp:2 * kp + 2, 128 * h:128 * (h + 1)]
                nc.tensor.matmul(
                    st[f"po{j}"][:],
                    lhsT,
                    wout_sb[:, 2 * kp:2 * kp + 2, 512 * j:512 * (j + 1)],
                    start=(kp == 0), stop=(kp == 3), perf_mode=DR)

            def emit_outproj_drain(h, j):
                st = state[h]
                if j == 0:
                    st["t3"] = ot_pool.tile([128, C], f32, name=f"t3_{h}",
                                            tag="ot")
                t1 = ot_pool.tile([128, 512], f32, tag="ot")
                nc.vector.tensor_scalar(t1[:], st[f"po{j}"][:], OUT_SCALE, None,
                                        ALU.mult)
                nc.vector.tensor_tensor(
                    out=st["t3"][:, 512 * j:512 * (j + 1)], in0=t1[:],
                    in1=xnb[:, h, 512 * j:512 * (j + 1)], op=ALU.add)
                if j == 1:
                    if h == H - 1:
                        # last tile: split across both queues to shorten the tail
                        nc.sync.dma_start(out=out_d[:, C * h:C * h + 512],
                                          in_=st["t3"][:, 0:512])
                        nc.gpsimd.dma_start(out=out_d[:, C * h + 512:C * (h + 1)],
                                            in_=st["t3"][:, 512:1024])
                    else:
                        eng = nc.sync if h % 2 == 0 else nc.gpsimd
                        eng.dma_start(out=out_d[:, C * h:C * (h + 1)],
                                      in_=st["t3"][:])
                    del state[h]

            # pipeline: head h does scores; h-1 attnV; h-2 out-proj
            for h in range(H + 2):
                if h < H:
                    emit_scores_start(h)
                for gk in range(NG):
                    if h < H:
                        emit_scores_gk(h, gk)
                    if h >= 1 and h - 1 < H:
                        emit_attnv_mm(h - 1, gk)
                    if h >= 2 and h - 2 < H:
                        emit_outproj_mm(h - 2, gk)
                        if gk == 3:
                            emit_outproj_drain(h - 2, 0)
                    # sums for pair p at gk = 2p+3 (exp of pair done ~1 iter ago)
                    if h < H and gk >= 3 and gk % 2 == 1:
                        emit_sums_pair(h, (gk - 3) // 2)
                if h < H:
                    emit_sums_pair(h, 3)
                    emit_recip(h)
                if h >= 1 and h - 1 < H:
                    emit_attnv_done(h - 1)
                if h >= 2 and h - 2 < H:
                    emit_outproj_drain(h - 2, 1)

        kT_pool.release()
        qT_pool.release()
        v_pool.release()

    return nc


_CACHE = {}


def _build(apply_affine: bool):
    key = apply_affine
    if key not in _CACHE:
        nc = bacc.Bacc("TRN2", target_bir_lowering=False, debug=False)
        _emit(nc, apply_affine)
        nc.compile()
        _CACHE[key] = nc
    return _CACHE[key]


def _to_fp8(a: np.ndarray) -> np.ndarray:
    return np.ascontiguousarray(
        np.clip(a, -FP8_MAX, FP8_MAX).astype(ml_dtypes.float8_e4m3))


def _pkn(w):
    """[C, n] row-major -> partition-major [128, (k, n)] with row c = 128k+p."""
    n = w.shape[1]
    return np.ascontiguousarray(
        w.reshape(NG, 128, n).transpose(1, 0, 2).reshape(128, NG * n))


def make_in_maps(inputs, B: int, apply_affine: bool):
    x = np.asarray(inputs["x"], np.float32)
    ln_g = np.asarray(inputs["ln_g"], np.float32)
    ln_b = np.asarray(inputs["ln_b"], np.float32)
    w_qkv = np.ascontiguousarray(np.asarray(inputs["w_qkv"], np.float32))
    b_qkv = np.asarray(inputs["b_qkv"], np.float32)
    w_out = np.ascontiguousarray(np.asarray(inputs["w_out"], np.float32))
    b_out = np.asarray(inputs["b_out"], np.float32)

    wqkv_fp8 = _to_fp8(w_qkv * SW)
    wq_pre = _pkn(wqkv_fp8[:, :C])
    wk_pre = _pkn(wqkv_fp8[:, C:2 * C])
    wv_pre = _pkn(wqkv_fp8[:, 2 * C:])
    wout_pre = _pkn(_to_fp8(w_out * SW))
    # q/k biases pre-scaled by SP (psum scale); [128, 16] col-per-slab
    bqk_pre = np.ascontiguousarray(
        (b_qkv[:2 * C] * SP).reshape(16, 128).T.astype(np.float32))
    # v-bias folded into out bias: softmax rows sum to 1
    b_out_eff = (b_out.astype(np.float64)
                 + b_qkv[2 * C:].astype(np.float64) @ w_out.astype(np.float64)
                 ).astype(np.float32)

    x_bf = x.astype(ml_dtypes.bfloat16)
    in_maps = []
    for c in range(B):
        m = {
            "x": _pkn(x_bf[c]),
            "w_q": wq_pre,
            "w_k": wk_pre,
            "w_v": wv_pre,
            "b_qk": bqk_pre,
            "w_out": wout_pre,
            "b_out_eff": b_out_eff,
        }
        if apply_affine:
            m["ln_g"] = ln_g
            m["ln_b"] = ln_b
        in_maps.append(m)
    return in_maps


def kernel(**inputs) -> np.ndarray:
    x = np.asarray(inputs["x"], np.float32)
    ln_g = np.asarray(inputs["ln_g"], np.float32)
    ln_b = np.asarray(inputs["ln_b"], np.float32)
    B = x.shape[0]
    assert x.shape == (B, L, C)
    apply_affine = not (np.all(ln_g == 1.0) and np.all(ln_b == 0.0))
    nc = _build(apply_affine)
    in_maps = make_in_maps(inputs, B, apply_affine)
    res = bass_utils.run_bass_kernel_spmd(nc, in_maps, core_ids=list(range(B)))
    # out is partition-major [128, (t, c)]: row l = 128t + p
    return np.stack([
        res.results[c]["out"].reshape(128, NT, C).transpose(1, 0, 2)
        .reshape(L, C) for c in range(B)
    ]).astype(np.float32)


# revision 54
# speedup vs baseline: 1.1920x; 1.0035x over previous
"""Trainium2 Bass kernel for an AttentionBlock (LN -> QKV -> attn -> out-proj + residual).

Shapes (hardcoded per problem spec): B=8, L=1024, C=1024, H=8 heads.
The reference uses a raw row-major reshape (torch-style .view) of q/k/v from
[B, L, C] to [B*H, L, C/H]; with L=1024, C=1024, H=8 this makes each
"attention head" operate on a contiguous 128-sequence-row block of the
[L, C] matrix, reinterpreted as [1024, 128].

Sharding: pure data-parallel over batch, one batch element per NeuronCore
(8 cores). No collectives.

Precision strategy (tolerance is 2e-2 relative; the attention path only
contributes ~1% of the output magnitude, the LN residual dominates):
  - x input and the LN residual xn in bf16.
  - All big matmuls (QKV proj, softmax-denominator sums, attn@V, out-proj)
    in fp8 e4m3 with DoubleRow (contract 256 per instruction, 2x PE rate).
  - Scores matmul in bf16 (K=128 per head-chunk, DoubleRow not applicable).
Scales (fp8 max is +-240 on TRN):
  - xnT = 16 * xn (SX); weights = 256 * w (SW); proj psum = 4096 (SP)
  - qT/kT = 4096 * q_true (bias pre-scaled on host); exp scale = S2/SP^2
  - v_fp8 = 32 * v_true (SV); attnT = 32 * attn_true (SA = SV)
  - out psum = 32*256 * attn@w_out -> final scale 1/8192
b_qkv's v-bias is folded into b_out on the host (softmax weights sum to 1).

DMA: total bandwidth ~220GB/s shared across the three DMA queues and
~max(22ns, bytes/150GBps) per descriptor row per queue, so x and out are
repacked on the host to partition-major layouts with contiguous 2-4KB rows,
x tiles load first (split across sync+gpsimd), weights follow in priority
order (wv, wq, wk, wout), and the scalar queue is left free so the LN
activations are not delayed behind DMA dispatch instructions.
"""

import math
from contextlib import ExitStack

import ml_dtypes
import numpy as np

import concourse.bass as bass
import concourse.bacc as bacc
import concourse.tile as tile
from concourse import mybir
from concourse import bass_utils
from concourse.masks import make_identity

L = 1024
C = 1024
H = 8          # heads; also number of 128-row l-tiles (head h <-> l-tile h)
CH = 128       # head dim
NT = 8         # l tiles (128 rows each)
NG = 8         # c groups (128 cols each)
EPS = 1e-5
S2 = 1.0 / math.sqrt(CH)   # combined q&k scale: (ch^-0.25)^2

SX = 16.0      # xn -> fp8 scale
SW = 256.0     # weight -> fp8 scale
SP = SX * SW   # projection psum scale
SV = 32.0      # v_fp8 = SV * v_true
SA = 32.0      # attnT = SA * attn_true (== SV so rb = recip directly)
OUT_SCALE = 1.0 / (SA * SW)
EXP_SCALE = S2 / (SP * SP)
FP8_MAX = 240.0

f32 = mybir.dt.float32
i32 = mybir.dt.int32
bf16 = mybir.dt.bfloat16
fp8 = mybir.dt.float8e4
# Schraudolph exp: exp(t) ~= bitcast_f32(int32(t*12102203.16 + 1064866805));
# +-3% systematic error that largely cancels in the softmax ratio.
SCH_A = 12102203.16
SCH_B = 1064866805.0
# (gk, j) half-tiles whose exp runs on DVE+gpsimd instead of ScalarE
# (ScalarE exp is the attention-phase floor; these positions have ~2.5us of
# slack before their sums-pair consumers)
DVE_EXP = {(2, 1), (4, 1)}
AF = mybir.ActivationFunctionType
ALU = mybir.AluOpType
DR = mybir.MatmulPerfMode.DoubleRow


def _bcast_ap(ap, p=128):
    """Broadcast a 1-D DRAM vector across p partitions (step-0 partition dim)."""
    return bass.AP(tensor=ap.tensor, offset=ap.offset, ap=[[0, p]] + list(ap.ap))


def _emit(nc, apply_affine: bool):
    # x and out are partition-major on the host: [p, t, c] with row l = 128t+p
    x_d = nc.dram_tensor("x", [128, NT * C], bf16, kind="ExternalInput").ap()
    wq_d = nc.dram_tensor("w_q", [128, NG * C], fp8, kind="ExternalInput").ap()
    wk_d = nc.dram_tensor("w_k", [128, NG * C], fp8, kind="ExternalInput").ap()
    wv_d = nc.dram_tensor("w_v", [128, NG * C], fp8, kind="ExternalInput").ap()
    bqk_d = nc.dram_tensor("b_qk", [128, 16], f32, kind="ExternalInput").ap()
    wout_d = nc.dram_tensor("w_out", [128, NG * C], fp8, kind="ExternalInput").ap()
    bout_d = nc.dram_tensor("b_out_eff", [C], f32, kind="ExternalInput").ap()
    if apply_affine:
        g_d = nc.dram_tensor("ln_g", [C], f32, kind="ExternalInput").ap()
        b_d = nc.dram_tensor("ln_b", [C], f32, kind="ExternalInput").ap()
    out_d = nc.dram_tensor("out", [128, NT * C], f32, kind="ExternalOutput").ap()

    with nc.allow_low_precision(reason="fp8/bf16 compute by design"), \
         tile.TileContext(nc) as tc, ExitStack() as ctx:
        const = ctx.enter_context(tc.tile_pool(name="const", bufs=1, side="left"))
        ident = const.tile([128, 128], bf16)
        make_identity(nc, ident)
        # DoubleRow lhsT needs the pair-dim stride to be a multiple of 16B
        ones2_t = const.tile([128, 2, 16], fp8)
        nc.vector.memset(ones2_t, 1.0)
        ones2 = ones2_t[:, :, 0:1]
        eps_sb = const.tile([128, 1], f32)
        nc.vector.memset(eps_sb, EPS)
        bqk_sb = const.tile([128, 16], f32)
        nc.sync.dma_start(out=bqk_sb[:], in_=bqk_d)
        bout_bc = const.tile([128, C], f32)
        if apply_affine:
            g_bc = const.tile([128, C], f32)
            b_bc = const.tile([128, C], f32)

        # Long-lived left-side tensors
        wqkv_pool = ctx.enter_context(tc.tile_pool(name="wqkv", bufs=1, side="left"))
        wv_sb = wqkv_pool.tile([128, NG, C], fp8)          # [c_in', k_in, d_v]
        wqk_sb = wqkv_pool.tile([128, NG, 2 * C], fp8)     # [c_in', k_in, d_qk]
        wout_pool = ctx.enter_context(tc.tile_pool(name="wout", bufs=1, side="left"))
        wout_sb = wout_pool.tile([128, NG, C], fp8)
        xt_pool = ctx.enter_context(tc.tile_pool(name="xt", bufs=1, side="left"))
        xt_all = xt_pool.tile([128, NT, C], bf16)   # raw x, [l_r, t, c]
        xn_pool = ctx.enter_context(tc.tile_pool(name="xn", bufs=1, side="left"))
        xn = xn_pool.tile([128, NT, C], bf16)       # normalized x
        xnb_pool = ctx.enter_context(tc.tile_pool(name="xnb", bufs=1, side="left"))
        xnb = xnb_pool.tile([128, NT, C], bf16)     # xn + b_out (residual+bias)
        xnT_pool = ctx.enter_context(tc.tile_pool(name="xnT", bufs=1, side="left"))
        xnT = xnT_pool.tile([128, NG, L], fp8)      # [c', g, l] = SX * xn.T
        attnT_pool = ctx.enter_context(tc.tile_pool(name="attnT", bufs=1, side="left"))
        attnT = attnT_pool.tile([128, NG, L], fp8)  # [c', g_q, l] = SA * attn.T
        v_pool = tc.alloc_tile_pool(name="v", bufs=1, side="left")
        v_fp8 = v_pool.tile([128, NT, C], fp8)      # [l_r, l-tile, c] = SV * v
        qT_pool = tc.alloc_tile_pool(name="qT", bufs=1, side="left")
        qT = qT_pool.tile([128, H, NG, 128], bf16)  # [c', h, g_q, l_r]
        kT_pool = tc.alloc_tile_pool(name="kT", bufs=1, side="left")
        kT = kT_pool.tile([128, NG, L], bf16)       # [c', g_k, l]

        # DMA plan: x tiles first (sync+gpsimd), then weights in priority
        # order split in k-halves across the same two queues. The scalar
        # queue carries no DMAs so LN activations issue immediately.
        xt_flat = xt_all[:].rearrange("p t c -> p (t c)")
        for t in range(NT):
            eng = nc.sync if t % 2 == 0 else nc.gpsimd
            eng.dma_start(out=xt_flat[:, C * t:C * (t + 1)],
                          in_=x_d[:, C * t:C * (t + 1)])

        def _load_w_halves(sb, dram, n):
            for half, eng in ((0, nc.sync), (1, nc.gpsimd)):
                eng.dma_start(
                    out=sb[:, 4 * half:4 * (half + 1), :],
                    in_=dram[:, 4 * n * half:4 * n * (half + 1)].rearrange(
                        "p (k n) -> p k n", k=4))

        _load_w_halves(wv_sb, wv_d, C)
        _load_w_halves(wqk_sb[:, :, 0:C], wq_d, C)
        _load_w_halves(wqk_sb[:, :, C:2 * C], wk_d, C)
        _load_w_halves(wout_sb, wout_d, C)
        if apply_affine:
            nc.gpsimd.dma_start(out=g_bc[:], in_=_bcast_ap(g_d))
            nc.gpsimd.dma_start(out=b_bc[:], in_=_bcast_ap(b_d))
        nc.gpsimd.dma_start(out=bout_bc[:], in_=_bcast_ap(bout_d))

        # ---------- Phase 1: per-tile LayerNorm + transpose to xnT ----------
        with tc.tile_pool(name="lnst", bufs=4, side="right") as lnst, \
             tc.tile_pool(name="lntmp", bufs=3, side="right") as lntmp, \
             tc.tile_pool(name="tr_ps", bufs=2, space="PSUM") as tr_ps, \
             tc.tile_pool(name="proj_ps", bufs=2, space="PSUM") as proj_ps:
            for t in range(NT):
                xt = xt_all[:, t, :]
                stats = lnst.tile([128, 2, 6], f32)
                for j in range(2):
                    nc.vector.bn_stats(out=stats[:, j, :],
                                       in_=xt[:, 512 * j:512 * (j + 1)])
                mv = lnst.tile([128, 2], f32)
                nc.vector.bn_aggr(out=mv[:], in_=stats[:])
                sq = lnst.tile([128, 1], f32)
                nc.scalar.activation(out=sq[:], in_=mv[:, 1:2], func=AF.Sqrt,
                                     bias=eps_sb[:], scale=1.0)
                rstd = lnst.tile([128, 1], f32)
                nc.vector.reciprocal(out=rstd[:], in_=sq[:])
                nmr = lnst.tile([128, 1], f32)
                nc.vector.tensor_scalar(nmr[:], mv[:, 0:1], rstd[:], -1.0,
                                        ALU.mult, ALU.mult)
                if apply_affine:
                    zt = lntmp.tile([128, C], f32)
                    nc.scalar.activation(out=zt[:], in_=xt, func=AF.Identity,
                                         bias=nmr[:], scale=rstd[:])
                    zg = lntmp.tile([128, C], f32)
                    nc.vector.tensor_tensor(out=zg[:], in0=zt[:], in1=g_bc[:],
                                            op=ALU.mult)
                    nc.vector.tensor_tensor(out=xn[:, t, :], in0=zg[:], in1=b_bc[:],
                                            op=ALU.add)
                else:
                    nc.scalar.activation(out=xn[:, t, :], in_=xt, func=AF.Identity,
                                         bias=nmr[:], scale=rstd[:])
                # transpose tile t: 8 PE transposes into one PSUM bank, then a
                # single DVE copy (x SX, cast to fp8) into xnT
                trp = tr_ps.tile([128, NG, 128], bf16, tag="tr")
                for g in range(NG):
                    nc.tensor.transpose(trp[:, g, :], xn[:, t, 128 * g:128 * (g + 1)],
                                        ident[:])
                nc.vector.tensor_scalar(
                    xnT[:, :, 128 * t:128 * (t + 1)], trp[:], SX, None, ALU.mult)

            # ---------- Phase 2: V projection (DoubleRow fp8) ----------
            for m in range(NT):
                psv = proj_ps.tile([128, C], f32, tag="proj")
                for kp in range(4):
                    lhsT = xnT[:, 2 * kp:2 * kp + 2, 128 * m:128 * (m + 1)]
                    for j in range(2):
                        nc.tensor.matmul(
                            psv[:, 512 * j:512 * (j + 1)], lhsT,
                            wv_sb[:, 2 * kp:2 * kp + 2, 512 * j:512 * (j + 1)],
                            start=(kp == 0), stop=(kp == 3), perf_mode=DR)
                nc.scalar.activation(out=v_fp8[:, m, :], in_=psv[:],
                                     func=AF.Identity, bias=0.0, scale=SV / SP)

            # ---------- Phase 3: Q, K projections (DoubleRow fp8) ----------
            for co in range(16):
                psq = proj_ps.tile([128, L], f32, tag="proj")
                for kp in range(4):
                    lhsT = wqk_sb[:, 2 * kp:2 * kp + 2, 128 * co:128 * (co + 1)]
                    for j in range(2):
                        nc.tensor.matmul(
                            psq[:, 512 * j:512 * (j + 1)], lhsT,
                            xnT[:, 2 * kp:2 * kp + 2, 512 * j:512 * (j + 1)],
                            start=(kp == 0), stop=(kp == 3), perf_mode=DR)
                bias_col = bqk_sb[:, co:co + 1]
                if co < 8:
                    # q: dst [c', h, l_r] over h (l = 128h + l_r); ScalarE
                    nc.scalar.activation(
                        out=qT[:, :, co, :],
                        in_=psq[:].rearrange("p (h l) -> p h l", h=H),
                        func=AF.Identity, bias=bias_col, scale=1.0)
                else:
                    # k: DVE
                    nc.vector.tensor_scalar(kT[:, co - 8, :], psq[:],
                                            bias_col, None, ALU.add)

            # residual + out-bias, precomputed so the out-proj drain is 2 ops
            for t in range(NT):
                nc.vector.tensor_tensor(out=xnb[:, t, :], in0=xn[:, t, :],
                                        in1=bout_bc[:], op=ALU.add)

        # ---------- Phase 4: attention + out-proj, software-pipelined ----------
        # stage lag: scores/exp/sums(h) | attnV(h-1) | out-proj(h-2)
        # PSUM budget (8 banks): scores 3x[128,512]=3, sums [1,L]=2,
        # attnV [128,L]=2, out-proj half [128,512]=1.
        with tc.tile_pool(name="pt", bufs=3, side="right") as pt_pool, \
             tc.tile_pool(name="rb", bufs=3, side="right") as rb_pool, \
             tc.tile_pool(name="recip", bufs=3, side="right") as recip_pool, \
             tc.tile_pool(name="ie", bufs=3, side="right") as ie_pool, \
             tc.tile_pool(name="ot", bufs=3, side="right") as ot_pool, \
             tc.tile_pool(name="s_ps", bufs=3, space="PSUM") as s_ps, \
             tc.tile_pool(name="sum_ps", bufs=1, space="PSUM") as sum_ps, \
             tc.tile_pool(name="av_ps", bufs=1, space="PSUM") as av_ps, \
             tc.tile_pool(name="o_ps", bufs=1, space="PSUM") as o_ps:

            state = {}   # per-head tiles carried across pipeline stages

            def emit_scores_start(h):
                pt = pt_pool.tile([128, NG, L], fp8, name=f"pt{h}", tag="pt")
                ps_sum = sum_ps.tile([1, L], f32, name=f"psum{h}", tag="ps_sum")
                state[h] = {"pt": pt, "ps_sum": ps_sum}

            def emit_scores_gk(h, gk):
                st = state[h]
                hs = slice(128 * h, 128 * (h + 1))
                qrow = qT[:, h, :, :].rearrange("p g l -> p (g l)")
                for j in range(2):
                    ps_s = s_ps.tile([128, 512], f32, tag="ps_s")
                    nc.tensor.matmul(ps_s[:], kT[:, gk, hs],
                                     qrow[:, 512 * j:512 * (j + 1)],
                                     start=True, stop=True)
                    dst = st["pt"][:, gk, 512 * j:512 * (j + 1)]
                    if (gk, j) in DVE_EXP:
                        ie = ie_pool.tile([128, 512], i32, tag="ie")
                        nc.vector.tensor_scalar(
                            ie[:], ps_s[:], EXP_SCALE * SCH_A, SCH_B,
                            ALU.mult, ALU.add)
                        nc.gpsimd.tensor_copy(dst, ie[:].bitcast(f32))
                    else:
                        nc.scalar.activation(out=dst, in_=ps_s[:], func=AF.Exp,
                                             bias=0.0, scale=EXP_SCALE)

            def emit_sums_pair(h, p):
                st = state[h]
                pt, ps_sum = st["pt"], st["ps_sum"]
                for j in range(2):
                    nc.tensor.matmul(
                        ps_sum[:, 512 * j:512 * (j + 1)], ones2,
                        pt[:, 2 * p:2 * p + 2, 512 * j:512 * (j + 1)],
                        start=(p == 0), stop=(p == 3), perf_mode=DR)

            def emit_recip(h):
                st = state[h]
                recip = recip_pool.tile([1, L], f32, tag="recip")
                nc.vector.reciprocal_approx_fast(out=recip[:], in_=st["ps_sum"])
                rb = rb_pool.tile([128, L], f32, tag="rb")
                nc.gpsimd.partition_broadcast(rb[:], recip[:])
                st["rb"] = rb

            def emit_attnv_mm(h, i):
                # i in 0..7 -> (p, j)
                st = state[h]
                p, j = divmod(i, 2)
                if i == 0:
                    st["av"] = av_ps.tile([128, L], f32, name=f"av{h}",
                                          tag="ps_av")
                vrow = v_fp8[:, h, :].rearrange("p (g c) -> p g c", g=NG)
                nc.tensor.matmul(
                    st["av"][:, 512 * j:512 * (j + 1)],
                    vrow[:, 2 * p:2 * p + 2, :],
                    st["pt"][:, 2 * p:2 * p + 2, 512 * j:512 * (j + 1)],
                    start=(p == 0), stop=(p == 3), perf_mode=DR)

            def emit_attnv_done(h):
                st = state[h]
                hs = slice(128 * h, 128 * (h + 1))
                nc.vector.tensor_tensor(
                    out=attnT[:, :, hs],
                    in0=st["av"][:].rearrange("p (g l) -> p g l", g=NG),
                    in1=st["rb"][:].rearrange("p (g l) -> p g l", g=NG),
                    op=ALU.mult)

            def emit_outproj_mm(h, i):
                # i in 0..7 -> (j, kp): j-major so each half finishes early
                st = state[h]
                j, kp = divmod(i, 4)
                if kp == 0:
                    st[f"po{j}"] = o_ps.tile([128, 512], f32, name=f"po{h}_{j}",
                                             tag="ps_o")
                lhsT = attnT[:, 2 * kp:2 * kp + 2, 128 * h:128 * (h + 1)]
                nc.tensor.matmul(
                    st[f"po{j}"][:],
                    lhsT,
                    wout_sb[:, 2 * kp:2 * kp + 2, 512 * j:512 * (j + 1)],
                    start=(kp == 0), stop=(kp == 3), perf_mode=DR)

            def emit_outproj_drain(h, j):
                st = state[h]
                if j == 0:
                    st["t3"] = ot_pool.tile([128, C], f32, name=f"t3_{h}",
                                            tag="ot")
                t1 = ot_pool.tile([128, 512], f32, tag="ot")
                nc.vector.tensor_scalar(t1[:], st[f"po{j}"][:], OUT_SCALE, None,
                                        ALU.mult)
                nc.vector.tensor_tensor(
                    out=st["t3"][:, 512 * j:512 * (j + 1)], in0=t1[:],
                    in1=xnb[:, h, 512 * j:512 * (j + 1)], op=ALU.add)
                if j == 1:
                    if h == H - 1:
                        # last tile: split across both queues to shorten the tail
                        nc.sync.dma_start(out=out_d[:, C * h:C * h + 512],
                                          in_=st["t3"][:, 0:512])
                        nc.gpsimd.dma_start(out=out_d[:, C * h + 512:C * (h + 1)],
                                            in_=st["t3"][:, 512:1024])
                    else:
                        eng = nc.sync if h % 2 == 0 else nc.gpsimd
                        eng.dma_start(out=out_d[:, C * h:C * (h + 1)],
                                      in_=st["t3"][:])
                    del state[h]

            # pipeline: head h does scores; h-1 attnV; h-2 out-proj
            for h in range(H + 2):
                if h < H:
                    emit_scores_start(h)
                for gk in range(NG):
                    if h < H:
                        emit_scores_gk(h, gk)
                    if h >= 1 and h - 1 < H:
                        emit_attnv_mm(h - 1, gk)
                    if h >= 2 and h - 2 < H:
                        emit_outproj_mm(h - 2, gk)
                        if gk == 3:
                            emit_outproj_drain(h - 2, 0)
                    # sums for pair p at gk = 2p+3 (exp of pair done ~1 iter ago)
                    if h < H and gk >= 3 and gk % 2 == 1:
                        emit_sums_pair(h, (gk - 3) // 2)
                if h < H:
                    emit_sums_pair(h, 3)
                    emit_recip(h)
                if h >= 1 and h - 1 < H:
                    emit_attnv_done(h - 1)
                if h >= 2 and h - 2 < H:
                    emit_outproj_drain(h - 2, 1)

        kT_pool.release()
        qT_pool.release()
        v_pool.release()

    return nc


_CACHE = {}


def _build(apply_affine: bool):
    key = apply_affine
    if key not in _CACHE:
        nc = bacc.Bacc("TRN2", target_bir_lowering=False, debug=False)
        _emit(nc, apply_affine)
        nc.compile()
        _CACHE[key] = nc
    return _CACHE[key]


def _to_fp8(a: np.ndarray) -> np.ndarray:
    return np.ascontiguousarray(
        np.clip(a, -FP8_MAX, FP8_MAX).astype(ml_dtypes.float8_e4m3))


def _pkn(w):
    """[C, n] row-major -> partition-major [128, (k, n)] with row c = 128k+p."""
    n = w.shape[1]
    return np.ascontiguousarray(
        w.reshape(NG, 128, n).transpose(1, 0, 2).reshape(128, NG * n))


def make_in_maps(inputs, B: int, apply_affine: bool):
    x = np.asarray(inputs["x"], np.float32)
    ln_g = np.asarray(inputs["ln_g"], np.float32)
    ln_b = np.asarray(inputs["ln_b"], np.float32)
    w_qkv = np.ascontiguousarray(np.asarray(inputs["w_qkv"], np.float32))
    b_qkv = np.asarray(inputs["b_qkv"], np.float32)
    w_out = np.ascontiguousarray(np.asarray(inputs["w_out"], np.float32))
    b_out = np.asarray(inputs["b_out"], np.float32)

    wqkv_fp8 = _to_fp8(w_qkv * SW)
    wq_pre = _pkn(wqkv_fp8[:, :C])
    wk_pre = _pkn(wqkv_fp8[:, C:2 * C])
    wv_pre = _pkn(wqkv_fp8[:, 2 * C:])
    wout_pre = _pkn(_to_fp8(w_out * SW))
    # q/k biases pre-scaled by SP (psum scale); [128, 16] col-per-slab
    bqk_pre = np.ascontiguousarray(
        (b_qkv[:2 * C] * SP).reshape(16, 128).T.astype(np.float32))
    # v-bias folded into out bias: softmax rows sum to 1
    b_out_eff = (b_out.astype(np.float64)
                 + b_qkv[2 * C:].astype(np.float64) @ w_out.astype(np.float64)
                 ).astype(np.float32)

    x_bf = x.astype(ml_dtypes.bfloat16)
    in_maps = []
    for c in range(B):
        m = {
            "x": _pkn(x_bf[c]),
            "w_q": wq_pre,
            "w_k": wk_pre,
            "w_v": wv_pre,
            "b_qk": bqk_pre,
            "w_out": wout_pre,
            "b_out_eff": b_out_eff,
        }
        if apply_affine:
            m["ln_g"] = ln_g
            m["ln_b"] = ln_b
        in_maps.append(m)
    return in_maps


def kernel(**inputs) -> np.ndarray:
    x = np.asarray(inputs["x"], np.float32)
    ln_g = np.asarray(inputs["ln_g"], np.float32)
    ln_b = np.asarray(inputs["ln_b"], np.float32)
    B = x.shape[0]
    assert x.shape == (B, L, C)
    apply_affine = not (np.all(ln_g == 1.0) and np.all(ln_b == 0.0))
    nc = _build(apply_affine)
    in_maps = make_in_maps(inputs, B, apply_affine)
    res = bass_utils.run_bass_kernel_spmd(nc, in_maps, core_ids=list(range(B)))
    # out is partition-major [128, (t, c)]: row l = 128t + p
    return np.stack([
        res.results[c]["out"].reshape(128, NT, C).transpose(1, 0, 2)
        .reshape(L, C) for c in range(B)
    ]).astype(np.float32)
